# revision 36
# baseline (speedup 1.0000x reference)
"""CPM3 attention kernel for 8 trn2 NeuronCores.

Sharding: tensor-parallel over heads (2 heads/core x both batches).
Device computes per-core partial outputs (Wo row-sharded); host sums.

Data layout tricks:
- host pre-transposes q/kv so the device never transposes big tensors;
  scores are computed transposed [k, q] so the softmax needs no
  partition-dim reductions (a ones-column in V yields the denominators).
- fp16 operands for all matmuls; PSUM accumulation stays fp32.
- softmax bias/mask enter MULTIPLICATIVELY: host precomputes
  E = exp(position_bias) * keep_mask, so p = exp(qk) * E.
- QK scores for (h0, h1) land in one 2-bank PSUM tile, so a single
  ACT instruction exps 1024 columns (amortizes ACT instruction
  overhead); PV runs 2 k-tiles behind QK (software pipeline).
- epilogue split: only the softmax normalization happens at q-tile
  boundaries; all output projections run as one dense tail block so
  they never stall the main-loop pipeline.
- PSUM budget: tag "sc" = 2 tiles x 2 banks, tag "ctx" = 2 tiles x 2
  banks; prologue/epilogue tiles reuse the same rings.
"""

import sys

sys.path.insert(0, "/opt/trn_rl_repo")

import numpy as np
import ml_dtypes

import concourse.bass as bass
import concourse.bacc as bacc
import concourse.tile as tile
import concourse.mybir as mybir
from concourse.bass_utils import run_bass_kernel_spmd

B, L, D, H, DH = 2, 2048, 1024, 16, 64
N_CORES = 8
HPC = H // N_CORES  # heads per core = 2
QTS = 512  # q tile size
QN = L // QTS  # 4
KP = 128  # k partition tile
KN = L // KP  # 16
KTG = 4  # k tiles per DMA group
KGN = KN // KTG  # 4
DC = D // 128  # 8 contraction chunks
HVW = 2 * (DH + 1)  # 130: hv_aug columns per k-tile (2 heads x (64+ones))
PV_LAG = 4  # k-tiles of software-pipeline distance for PV

F32 = mybir.dt.float32
F32R = mybir.dt.float32r
F16 = mybir.dt.float16

_CACHE: dict = {}


def _build():
    if "nc" in _CACHE:
        return _CACHE["nc"]
    nc = bacc.Bacc("TRN2", target_bir_lowering=False, debug=False, num_devices=N_CORES)

    qT = nc.dram_tensor("qT", [B, DC, 128, L], F16, kind="ExternalInput").ap()
    kvT = nc.dram_tensor("kvT", [B, DC, 128, L], F16, kind="ExternalInput").ap()
    wq = nc.dram_tensor("wq", [128, DC, 128], F16, kind="ExternalInput").ap()
    wk = nc.dram_tensor("wk", [128, DC, 128], F16, kind="ExternalInput").ap()
    wv = nc.dram_tensor("wv", [128, DC, 128], F16, kind="ExternalInput").ap()
    wo = nc.dram_tensor("wo", [128, D], F16, kind="ExternalInput").ap()
    eb = nc.dram_tensor(
        "eb", [QN, KGN, 128, KTG, B, HPC, QTS], F16, kind="ExternalInput"
    ).ap()
    identr = nc.dram_tensor("identr", [128, 128], F32R, kind="ExternalInput").ap()
    indh = nc.dram_tensor("indh", [1, 256], F16, kind="ExternalInput").ap()
    out = nc.dram_tensor("out", [B, L, D], F16, kind="ExternalOutput").ap()

    with tile.TileContext(nc) as tc:
        with (
            tc.tile_pool(name="const", bufs=1) as constp,
            tc.tile_pool(name="hq", bufs=2) as hqp,
            tc.tile_pool(name="hk", bufs=2) as hkp,
            tc.tile_pool(name="hv", bufs=2) as hvp,
            tc.tile_pool(name="stage", bufs=3) as stagep,
            tc.tile_pool(name="ebp", bufs=2) as ebp,
            tc.tile_pool(name="p1", bufs=6) as p1p,
            tc.tile_pool(name="pt", bufs=14) as ptp,
            tc.tile_pool(name="ctxn", bufs=2 * QN) as ctxnp,
            tc.tile_pool(name="rc", bufs=4) as rcp,
            tc.tile_pool(name="outb", bufs=4) as outp,
            tc.tile_pool(name="psum", bufs=2, space=bass.MemorySpace.PSUM) as psp,
        ):
            # ---- constants (wq/wk first: the prologue blocks on them) ----
            wq_t = constp.tile([128, DC, 128], F16, tag="wq")
            nc.sync.dma_start(wq_t[:], wq[:])
            wk_t = constp.tile([128, DC, 128], F16, tag="wk")
            nc.scalar.dma_start(wk_t[:], wk[:])
            wv_t = constp.tile([128, DC, 128], F16, tag="wv")
            nc.scalar.dma_start(wv_t[:], wv[:])
            identr_t = constp.tile([128, 128], F32R, tag="identr")
            nc.sync.dma_start(identr_t[:], identr[:])
            indh_t = constp.tile([1, 256], F16, tag="indh")
            nc.sync.dma_start(indh_t[:], indh[:])
            wo_t = constp.tile([128, D], F16, tag="wo")
            nc.sync.dma_start(wo_t[:], wo[:])

            # ---- prologue: projections ----
            # q chunks on the sync queue, kv chunks on the scalar queue (more
            # SDMA parallelism); each [128, L] chunk is loaded as two halves
            # so matmuls start after the first half arrives.
            hq_sb, hk_sb, hv_sb, hvT = {}, {}, {}, {}
            hq_ps, hk_ps, hv_ps = {}, {}, {}

            def half_dma(engines, dst, src):
                engines[0].dma_start(dst[:, 0 : L // 2], src[:, 0 : L // 2])
                engines[1].dma_start(dst[:, L // 2 : L], src[:, L // 2 : L])

            for b in range(B):
                # -- projection matmuls for batch b --
                hq_ps[b] = [
                    psp.tile([128, 2, QTS], F32, tag="sc", name=f"hq_ps{b}_{i}")
                    for i in range(2)
                ]
                for dc in range(DC):
                    qc = stagep.tile([128, L], F16, tag="stage", name=f"qc{b}_{dc}")
                    half_dma((nc.sync, nc.sync), qc, qT[b, dc])
                    for qt in range(QN):
                        nc.tensor.matmul(
                            hq_ps[b][qt // 2][:, qt % 2, :],
                            wq_t[:, dc, :],
                            qc[:, qt * QTS : (qt + 1) * QTS],
                            start=(dc == 0),
                            stop=(dc == DC - 1),
                        )
                hq_sb[b] = hqp.tile([128, L], F16, tag="hq", name=f"hq_sb{b}")
                for qt in range(QN):
                    nc.scalar.copy(
                        hq_sb[b][:, qt * QTS : (qt + 1) * QTS],
                        hq_ps[b][qt // 2][:, qt % 2, :],
                    )

                hk_ps[b] = [
                    psp.tile([128, 2, QTS], F32, tag="sc", name=f"hk_ps{b}_{i}")
                    for i in range(2)
                ]
                hv_ps[b] = [
                    psp.tile([128, 2, QTS], F32, tag="ctx", name=f"hv_ps{b}_{i}")
                    for i in range(2)
                ]
                for dc in range(DC):
                    kc = stagep.tile([128, L], F16, tag="stage", name=f"kc{b}_{dc}")
                    half_dma((nc.scalar, nc.gpsimd), kc, kvT[b, dc])
                    for qt in range(QN):
                        nc.tensor.matmul(
                            hk_ps[b][qt // 2][:, qt % 2, :],
                            wk_t[:, dc, :],
                            kc[:, qt * QTS : (qt + 1) * QTS],
                            start=(dc == 0),
                            stop=(dc == DC - 1),
                        )
                        nc.tensor.matmul(
                            hv_ps[b][qt // 2][:, qt % 2, :],
                            wv_t[:, dc, :],
                            kc[:, qt * QTS : (qt + 1) * QTS],
                            start=(dc == 0),
                            stop=(dc == DC - 1),
                        )
                hk_sb[b] = hkp.tile([128, L], F16, tag="hk", name=f"hk_sb{b}")
                hvT[b] = stagep.tile([128, L], F32R, tag="stage", name=f"hvT{b}")
                for qt in range(QN):
                    nc.scalar.copy(
                        hk_sb[b][:, qt * QTS : (qt + 1) * QTS],
                        hk_ps[b][qt // 2][:, qt % 2, :],
                    )
                    nc.vector.tensor_copy(
                        hvT[b][:, qt * QTS : (qt + 1) * QTS],
                        hv_ps[b][qt // 2][:, qt % 2, :],
                    )

            # E prefetch: issued only now so the projection DMAs get the
            # full HBM bandwidth first
            pre_eb = ebp.tile([128, KTG, B, HPC, QTS], F16, tag="eb", name="pre_eb")
            nc.gpsimd.dma_start(pre_eb[:], eb[0, 0])

            # -- hv_aug: transpose hvT per k-tile; ones cols prefilled --
            for b in range(B):
                hv_sb[b] = hvp.tile(
                    [128, KN * HVW + 64], F16, tag="hv", name=f"hv_sb{b}"
                )
                nc.gpsimd.memset(hv_sb[b][:].bitcast(mybir.dt.uint16), 0x3C00)
            for b in range(B):
                for kt in range(KN):
                    tp = psp.tile([128, 128], F32R, tag="sc")
                    nc.tensor.transpose(
                        tp[:], hvT[b][:, kt * KP : (kt + 1) * KP], identr_t[:]
                    )
                    o = kt * HVW
                    nc.vector.tensor_copy(hv_sb[b][:, o : o + DH], tp[:, 0:DH])
                    nc.vector.tensor_copy(
                        hv_sb[b][:, o + DH + 1 : o + 2 * DH + 1], tp[:, DH:128]
                    )

            # ---- normalize: softmax denominators -> ctxn (fp16) ----
            ctxn_sb = {}

            def emit_normalize(qt, ctx_ps):
                for b in range(B):
                    ctxn = ctxnp.tile(
                        [128, QTS], F16, tag="ctxn", name=f"ctxn{b}_{qt}"
                    )
                    ctxn_sb[(b, qt)] = ctxn
                    bcw = psp.tile([128, 2, QTS], F32, tag="sc", name=f"bcw{b}_{qt}")
                    bc = bcw[:, 0, :]
                    for h in range(HPC):
                        dsb = rcp.tile(
                            [1, QTS], F32, tag="dsb", name=f"dsb{b}_{h}_{qt}"
                        )
                        nc.vector.tensor_copy(dsb[:], ctx_ps[b][DH : DH + 1, h, :])
                        rcf = rcp.tile(
                            [1, QTS], F32, tag="rcf", name=f"rcf{b}_{h}_{qt}"
                        )
                        nc.vector.reciprocal_approx_fast(rcf[:], dsb[:])
                        rcr = rcp.tile(
                            [1, QTS], F16, tag="rcr", name=f"rcr{b}_{h}_{qt}"
                        )
                        nc.vector.tensor_copy(rcr[:], rcf[:])
                        nc.tensor.matmul(
                            bc,
                            indh_t[:, h * 128 : (h + 1) * 128],
                            rcr[:],
                            start=(h == 0),
                            stop=(h == HPC - 1),
                        )
                    bc_sb = rcp.tile([128, QTS], F32, tag="bcsb", name=f"bc_sb{b}_{qt}")
                    nc.vector.tensor_copy(bc_sb[:], bc)
                    for h in range(HPC):
                        nc.vector.tensor_tensor(
                            ctxn[h * DH : (h + 1) * DH, :],
                            ctx_ps[b][0:DH, h, :],
                            bc_sb[h * DH : (h + 1) * DH, :],
                            mybir.AluOpType.mult,
                        )

            # ---- main loop ----
            ctx_map = {}  # qt -> {b: [128, HPC, QTS] psum ap}
            pending_pv = []  # groups of [(b, h, kt, p_t, qt)], oldest first
            pending_norm = None

            def flush_pv_group():
                group = pending_pv.pop(0)
                for b, h, pkt, p_ap, pqt in group:
                    o = pkt * HVW + h * (DH + 1)
                    nc.tensor.matmul(
                        ctx_map[pqt][b][:, h, :],
                        hv_sb[b][:, o : o + 128],
                        p_ap,
                        start=(pkt == 0),
                        stop=(pkt == KN - 1),
                    )

            out_parts = []  # deferred output-projection closures

            def emit_out_part(b, pqt, qs, oh, tail=False):
                # half-size part: ONE matmul + ONE [128,512] drain keeps the
                # borrowed sc-ring slot occupancy short and fits the DVE
                # per-k-tile budget
                ctxn = ctxn_sb[(b, pqt)]
                ob = outp.tile(
                    [128, QTS], F16, tag="outb", name=f"ob{b}_{qs}_{oh}_{pqt}"
                )
                op_ps = psp.tile(
                    [128, 2, QTS], F32, tag="sc", name=f"op{b}_{qs}_{oh}_{pqt}"
                )
                nc.tensor.matmul(
                    op_ps[:, 0, :],
                    ctxn[:, qs * 128 : (qs + 1) * 128],
                    wo_t[:, oh * QTS : (oh + 1) * QTS],
                    start=True,
                    stop=True,
                )
                if tail and oh == 1:
                    # ACT is idle at the tail; split drain + DMA queues
                    nc.scalar.copy(ob[:], op_ps[:, 0, :])
                    dq = nc.scalar
                else:
                    nc.vector.tensor_copy(ob[:], op_ps[:, 0, :])
                    dq = nc.sync
                r0 = pqt * QTS + qs * 128
                dq.dma_start(
                    out[b, r0 : r0 + 128, oh * QTS : (oh + 1) * QTS], ob[:]
                )

            def emit_qk(qt_, kt_):
                # QK: (h0, h1) into a 2-bank PSUM tile (h0 rows 0-63 /
                # h1 rows 64-127 also land on different PE row-tiles)
                sc_ = {}
                for b in range(B):
                    sc_[b] = psp.tile(
                        [128, HPC, QTS],
                        F32,
                        tag="sc",
                        name=f"sc{b}_{kt_}_{qt_}",
                    )
                    for h in range(HPC):
                        nc.tensor.matmul(
                            sc_[b][:, h, :],
                            hk_sb[b][
                                h * DH : (h + 1) * DH, kt_ * KP : (kt_ + 1) * KP
                            ],
                            hq_sb[b][
                                h * DH : (h + 1) * DH,
                                qt_ * QTS : (qt_ + 1) * QTS,
                            ],
                            start=True,
                            stop=True,
                        )
                return sc_

            # QK emission runs one k-tile AHEAD of exp/PV emission, so each
            # next tile's QK sits in the PE queue BEFORE the current tile's
            # PV flush and output-projection matmuls: the exp stream never
            # waits on tail-of-tile PE work.
            tl = [(qt, kt) for qt in range(QN) for kt in range(KN)]
            eb_cur = None
            sc_cur = None
            for idx, (qt, kt) in enumerate(tl):
                kg, ki = kt // KTG, kt % KTG
                just_norm = False
                if kt == 0:
                    ctx_map[qt] = {
                        bb: psp.tile(
                            [128, HPC, QTS],
                            F32,
                            tag="ctx",
                            name=f"ctx_ps{bb}_{qt}",
                        )
                        for bb in range(B)
                    }
                if ki == 0:
                    if qt == 0 and kg == 0:
                        eb_cur = pre_eb
                    else:
                        eb_cur = ebp.tile(
                            [128, KTG, B, HPC, QTS], F16, tag="eb", name=f"eb_t{qt}_{kg}"
                        )
                        nc.gpsimd.dma_start(eb_cur[:], eb[qt, kg])
                if idx == 0:
                    sc_cur = emit_qk(qt, kt)
                sc_nxt = emit_qk(*tl[idx + 1]) if idx + 1 < len(tl) else None
                # both batches' probs in one tile: 2 exps, ONE wide
                # fp16 multiply (DVE 2x mode over 2048 columns)
                p1_t = p1p.tile(
                    [128, B * HPC, QTS], F16, tag="p1", name=f"p1_{kt}_{qt}"
                )
                for b in range(B):
                    nc.scalar.activation(
                        p1_t[:, b * HPC : (b + 1) * HPC, :],
                        sc_cur[b][:],
                        mybir.ActivationFunctionType.Exp,
                    )
                p2 = ptp.tile(
                    [128, B * HPC, QTS],
                    F16,
                    tag="pt2",
                    bufs=7,
                    name=f"p2_{kt}_{qt}",
                )
                nc.vector.tensor_tensor(
                    p2[:],
                    p1_t[:],
                    eb_cur[:, ki, :, :, :],
                    mybir.AluOpType.mult,
                )
                new_group = []
                for b in range(B):
                    for h in range(HPC):
                        new_group.append((b, h, kt, p2[:, b * HPC + h, :], qt))
                pending_pv.append(new_group)
                sc_cur = sc_nxt
                if kt == PV_LAG and pending_norm is not None:
                    # all of qt-1's PVs have flushed; normalize it (frees
                    # its ctx banks) and queue its output projections
                    pqt = pending_norm[0]
                    emit_normalize(*pending_norm)
                    pending_norm = None
                    for bb in range(B):
                        for qs in range(QN):
                            for oh in range(2):
                                out_parts.append((bb, pqt, qs, oh))
                    just_norm = True
                if just_norm:
                    pass  # skip one flush beat: give the normalize
                    # chain time before the first new-ctx PV
                else:
                    while len(pending_pv) > PV_LAG:
                        flush_pv_group()
                if out_parts and kt >= PV_LAG + 1:
                    emit_out_part(*out_parts.pop(0))
                    if len(out_parts) > 10:
                        emit_out_part(*out_parts.pop(0))
                if kt == KN - 1:
                    pending_norm = (qt, ctx_map[qt])
            while pending_pv:
                flush_pv_group()
            pqt = pending_norm[0]
            emit_normalize(*pending_norm)
            for bb in range(B):
                for qs in range(QN):
                    for oh in range(2):
                        out_parts.append((bb, pqt, qs, oh))
            while out_parts:
                emit_out_part(*out_parts.pop(0), tail=True)

    nc.compile()
    _CACHE["nc"] = nc
    return nc


def _prep_core(core, position_bias, Wq, Wk, Wv, Wo, shared):
    """Per-core input map. `shared` holds core-independent packed arrays."""
    h0 = core * HPC
    rows = slice(h0 * DH, (h0 + HPC) * DH)

    def packw(w, scale=1.0):
        return np.ascontiguousarray(
            (w[rows].T * scale).reshape(DC, 128, 128).transpose(1, 0, 2)
        ).astype(np.float16)

    # E = exp(pb) * keep, per (b, h) -> [qt, kg, kp, ktg, b, h, qf]
    ecomb = shared["epb"][h0 : h0 + HPC][None] * shared["keep"][:, None]  # [B,HPC,q,k]
    ebp = np.ascontiguousarray(
        ecomb.reshape(B, HPC, QN, QTS, KGN, KTG, 128).transpose(2, 4, 6, 5, 0, 1, 3)
    ).astype(np.float16)
    return {
        "qT": shared["qT"],
        "kvT": shared["kvT"],
        "identr": shared["identr"],
        "indh": shared["indh"],
        "wq": packw(Wq, 1.0 / np.sqrt(DH)),
        "wk": packw(Wk),
        "wv": packw(Wv),
        "wo": np.ascontiguousarray(Wo[:, rows].T).astype(np.float16),
        "eb": ebp,
    }


def _prep_shared(query, key_value, mask, position_bias):
    qTp = np.ascontiguousarray(
        query.reshape(B, L, DC, 128).transpose(0, 2, 3, 1)
    ).astype(np.float16)
    kvTp = np.ascontiguousarray(
        key_value.reshape(B, L, DC, 128).transpose(0, 2, 3, 1)
    ).astype(np.float16)
    epb = np.exp(position_bias, dtype=np.float32)  # [H, q, k]
    keep = np.asarray(mask, dtype=np.float32)  # [B, q, k] 1=keep
    indh = np.concatenate(
        [
            np.where(np.arange(128) < 64, 1.0, 0.0),
            np.where(np.arange(128) >= 64, 1.0, 0.0),
        ]
    ).astype(np.float16)[None, :]
    return {
        "qT": qTp,
        "kvT": kvTp,
        "epb": epb,
        "keep": keep,
        "identr": np.eye(128, dtype=np.float32),
        "indh": np.ascontiguousarray(indh),
    }


def kernel(query, key_value, mask, position_bias, Wq, Wk, Wv, Wo, _trace=False):
    query = np.asarray(query, dtype=np.float32)
    key_value = np.asarray(key_value, dtype=np.float32)
    mask = np.asarray(mask)
    position_bias = np.asarray(position_bias, dtype=np.float32)
    Wq = np.asarray(Wq, dtype=np.float32)
    Wk = np.asarray(Wk, dtype=np.float32)
    Wv = np.asarray(Wv, dtype=np.float32)
    Wo = np.asarray(Wo, dtype=np.float32)

    nc = _build()
    shared = _prep_shared(query, key_value, mask, position_bias)
    in_maps = [
        _prep_core(c, position_bias, Wq, Wk, Wv, Wo, shared) for c in range(N_CORES)
    ]
    res = run_bass_kernel_spmd(nc, in_maps, list(range(N_CORES)), trace=_trace)
    _CACHE["last_result"] = res
    acc = res.results[0]["out"].astype(np.float64)
    for c in range(1, N_CORES):
        acc += res.results[c]["out"]
    return acc.astype(np.float32)


# revision 37
# speedup vs baseline: 1.0768x; 1.0768x over previous
"""CPM3 attention kernel for 8 trn2 NeuronCores.

Sharding: tensor-parallel over heads (2 heads/core x both batches).
Device computes per-core partial outputs (Wo row-sharded); host sums.

Data layout tricks:
- host pre-transposes q/kv so the device never transposes big tensors;
  scores are computed transposed [k, q] so the softmax needs no
  partition-dim reductions (a ones-column in V yields the denominators).
- fp16 operands for all matmuls; PSUM accumulation stays fp32.
- softmax bias/mask enter MULTIPLICATIVELY: host precomputes
  E = exp(position_bias) * keep_mask, so p = exp(qk) * E.
- QK scores for (h0, h1) land in one 2-bank PSUM tile, so a single
  ACT instruction exps 1024 columns (amortizes ACT instruction
  overhead); PV runs 2 k-tiles behind QK (software pipeline).
- epilogue split: only the softmax normalization happens at q-tile
  boundaries; all output projections run as one dense tail block so
  they never stall the main-loop pipeline.
- PSUM budget: tag "sc" = 2 tiles x 2 banks, tag "ctx" = 2 tiles x 2
  banks; prologue/epilogue tiles reuse the same rings.
"""

import sys

sys.path.insert(0, "/opt/trn_rl_repo")

import numpy as np
import ml_dtypes

import concourse.bass as bass
import concourse.bacc as bacc
import concourse.tile as tile
import concourse.mybir as mybir
from concourse.bass_utils import run_bass_kernel_spmd

B, L, D, H, DH = 2, 2048, 1024, 16, 64
N_CORES = 8
HPC = H // N_CORES  # heads per core = 2
QTS = 512  # q tile size
QN = L // QTS  # 4
KP = 128  # k partition tile
KN = L // KP  # 16
KTG = 4  # k tiles per DMA group
KGN = KN // KTG  # 4
DC = D // 128  # 8 contraction chunks
HVW = 2 * (DH + 1)  # 130: hv_aug columns per k-tile (2 heads x (64+ones))
PV_LAG = 4  # k-tiles of software-pipeline distance for PV

F32 = mybir.dt.float32
F32R = mybir.dt.float32r
F16 = mybir.dt.float16

_CACHE: dict = {}


def _build():
    if "nc" in _CACHE:
        return _CACHE["nc"]
    nc = bacc.Bacc("TRN2", target_bir_lowering=False, debug=False, num_devices=N_CORES)

    qT = nc.dram_tensor("qT", [B, DC, 128, L], F16, kind="ExternalInput").ap()
    kvT = nc.dram_tensor("kvT", [B, DC, 128, L], F16, kind="ExternalInput").ap()
    wq = nc.dram_tensor("wq", [128, DC, 128], F16, kind="ExternalInput").ap()
    wk = nc.dram_tensor("wk", [128, DC, 128], F16, kind="ExternalInput").ap()
    wv = nc.dram_tensor("wv", [128, DC, 128], F16, kind="ExternalInput").ap()
    wo = nc.dram_tensor("wo", [128, D], F16, kind="ExternalInput").ap()
    eb = nc.dram_tensor(
        "eb", [QN, KGN, 128, KTG, B, HPC, QTS], F16, kind="ExternalInput"
    ).ap()
    identr = nc.dram_tensor("identr", [128, 128], F32R, kind="ExternalInput").ap()
    indh = nc.dram_tensor("indh", [1, 256], F16, kind="ExternalInput").ap()
    out = nc.dram_tensor("out", [B, L, D], F16, kind="ExternalOutput").ap()

    with tile.TileContext(nc) as tc:
        with (
            tc.tile_pool(name="const", bufs=1) as constp,
            tc.tile_pool(name="hq", bufs=2) as hqp,
            tc.tile_pool(name="hk", bufs=2) as hkp,
            tc.tile_pool(name="hv", bufs=2) as hvp,
            tc.tile_pool(name="stage", bufs=3) as stagep,
            tc.tile_pool(name="ebp", bufs=2) as ebp,
            tc.tile_pool(name="p1", bufs=6) as p1p,
            tc.tile_pool(name="pt", bufs=14) as ptp,
            tc.tile_pool(name="ctxn", bufs=2 * QN) as ctxnp,
            tc.tile_pool(name="rc", bufs=4) as rcp,
            tc.tile_pool(name="outb", bufs=4) as outp,
            tc.tile_pool(name="psum", bufs=2, space=bass.MemorySpace.PSUM) as psp,
        ):
            # ---- constants (wq/wk first: the prologue blocks on them) ----
            wq_t = constp.tile([128, DC, 128], F16, tag="wq")
            nc.sync.dma_start(wq_t[:], wq[:])
            wk_t = constp.tile([128, DC, 128], F16, tag="wk")
            nc.scalar.dma_start(wk_t[:], wk[:])
            wv_t = constp.tile([128, DC, 128], F16, tag="wv")
            nc.scalar.dma_start(wv_t[:], wv[:])
            identr_t = constp.tile([128, 128], F32R, tag="identr")
            nc.sync.dma_start(identr_t[:], identr[:])
            indh_t = constp.tile([1, 256], F16, tag="indh")
            nc.sync.dma_start(indh_t[:], indh[:])
            wo_t = constp.tile([128, D], F16, tag="wo")
            nc.sync.dma_start(wo_t[:], wo[:])

            # ---- prologue: projections ----
            # q chunks on the sync queue, kv chunks on the scalar queue (more
            # SDMA parallelism); each [128, L] chunk is loaded as two halves
            # so matmuls start after the first half arrives.
            hq_sb, hk_sb, hv_sb, hvT = {}, {}, {}, {}
            hq_ps, hk_ps, hv_ps = {}, {}, {}

            def half_dma(engines, dst, src):
                engines[0].dma_start(dst[:, 0 : L // 2], src[:, 0 : L // 2])
                engines[1].dma_start(dst[:, L // 2 : L], src[:, L // 2 : L])

            for b in range(B):
                # -- projection matmuls for batch b --
                hq_ps[b] = [
                    psp.tile([128, 2, QTS], F32, tag="sc", name=f"hq_ps{b}_{i}")
                    for i in range(2)
                ]
                for dc in range(DC):
                    qc = stagep.tile([128, L], F16, tag="stage", name=f"qc{b}_{dc}")
                    half_dma((nc.sync, nc.sync), qc, qT[b, dc])
                    for qt in range(QN):
                        nc.tensor.matmul(
                            hq_ps[b][qt // 2][:, qt % 2, :],
                            wq_t[:, dc, :],
                            qc[:, qt * QTS : (qt + 1) * QTS],
                            start=(dc == 0),
                            stop=(dc == DC - 1),
                        )
                hq_sb[b] = hqp.tile([128, L], F16, tag="hq", name=f"hq_sb{b}")
                for qt in range(QN):
                    nc.scalar.copy(
                        hq_sb[b][:, qt * QTS : (qt + 1) * QTS],
                        hq_ps[b][qt // 2][:, qt % 2, :],
                    )

                hk_ps[b] = [
                    psp.tile([128, 2, QTS], F32, tag="sc", name=f"hk_ps{b}_{i}")
                    for i in range(2)
                ]
                hv_ps[b] = [
                    psp.tile([128, 2, QTS], F32, tag="ctx", name=f"hv_ps{b}_{i}")
                    for i in range(2)
                ]
                for dc in range(DC):
                    kc = stagep.tile([128, L], F16, tag="stage", name=f"kc{b}_{dc}")
                    half_dma((nc.scalar, nc.gpsimd), kc, kvT[b, dc])
                    for qt in range(QN):
                        nc.tensor.matmul(
                            hk_ps[b][qt // 2][:, qt % 2, :],
                            wk_t[:, dc, :],
                            kc[:, qt * QTS : (qt + 1) * QTS],
                            start=(dc == 0),
                            stop=(dc == DC - 1),
                        )
                        nc.tensor.matmul(
                            hv_ps[b][qt // 2][:, qt % 2, :],
                            wv_t[:, dc, :],
                            kc[:, qt * QTS : (qt + 1) * QTS],
                            start=(dc == 0),
                            stop=(dc == DC - 1),
                        )
                hk_sb[b] = hkp.tile([128, L], F16, tag="hk", name=f"hk_sb{b}")
                hvT[b] = stagep.tile([128, L], F32R, tag="stage", name=f"hvT{b}")
                for qt in range(QN):
                    nc.scalar.copy(
                        hk_sb[b][:, qt * QTS : (qt + 1) * QTS],
                        hk_ps[b][qt // 2][:, qt % 2, :],
                    )
                    nc.vector.tensor_copy(
                        hvT[b][:, qt * QTS : (qt + 1) * QTS],
                        hv_ps[b][qt // 2][:, qt % 2, :],
                    )

            # E prefetch: issued only now so the projection DMAs get the
            # full HBM bandwidth first
            pre_eb = ebp.tile([128, KTG, B, HPC, QTS], F16, tag="eb", name="pre_eb")
            nc.gpsimd.dma_start(pre_eb[:], eb[0, 0])

            # -- hv_aug: transpose hvT per k-tile; ones cols prefilled --
            for b in range(B):
                hv_sb[b] = hvp.tile(
                    [128, KN * HVW + 64], F16, tag="hv", name=f"hv_sb{b}"
                )
                nc.gpsimd.memset(hv_sb[b][:].bitcast(mybir.dt.uint16), 0x3C00)
            for b in range(B):
                for kt in range(KN):
                    tp = psp.tile([128, 128], F32R, tag="sc")
                    nc.tensor.transpose(
                        tp[:], hvT[b][:, kt * KP : (kt + 1) * KP], identr_t[:]
                    )
                    o = kt * HVW
                    nc.vector.tensor_copy(hv_sb[b][:, o : o + DH], tp[:, 0:DH])
                    nc.vector.tensor_copy(
                        hv_sb[b][:, o + DH + 1 : o + 2 * DH + 1], tp[:, DH:128]
                    )

            # ---- normalize: softmax denominators -> ctxn (fp16) ----
            ctxn_sb = {}

            def emit_normalize(qt, ctx_ps):
                for b in range(B):
                    ctxn = ctxnp.tile(
                        [128, QTS], F16, tag="ctxn", name=f"ctxn{b}_{qt}"
                    )
                    ctxn_sb[(b, qt)] = ctxn
                    bcw = psp.tile([128, 2, QTS], F32, tag="sc", name=f"bcw{b}_{qt}")
                    bc = bcw[:, 0, :]
                    for h in range(HPC):
                        dsb = rcp.tile(
                            [1, QTS], F32, tag="dsb", name=f"dsb{b}_{h}_{qt}"
                        )
                        nc.vector.tensor_copy(dsb[:], ctx_ps[b][DH : DH + 1, h, :])
                        rcf = rcp.tile(
                            [1, QTS], F32, tag="rcf", name=f"rcf{b}_{h}_{qt}"
                        )
                        nc.vector.reciprocal_approx_fast(rcf[:], dsb[:])
                        rcr = rcp.tile(
                            [1, QTS], F16, tag="rcr", name=f"rcr{b}_{h}_{qt}"
                        )
                        nc.vector.tensor_copy(rcr[:], rcf[:])
                        nc.tensor.matmul(
                            bc,
                            indh_t[:, h * 128 : (h + 1) * 128],
                            rcr[:],
                            start=(h == 0),
                            stop=(h == HPC - 1),
                        )
                    bc_sb = rcp.tile([128, QTS], F32, tag="bcsb", name=f"bc_sb{b}_{qt}")
                    nc.vector.tensor_copy(bc_sb[:], bc)
                    for h in range(HPC):
                        nc.vector.tensor_tensor(
                            ctxn[h * DH : (h + 1) * DH, :],
                            ctx_ps[b][0:DH, h, :],
                            bc_sb[h * DH : (h + 1) * DH, :],
                            mybir.AluOpType.mult,
                        )

            # ---- main loop ----
            ctx_map = {}  # qt -> {b: [128, HPC, QTS] psum ap}
            pending_pv = []  # groups of [(b, h, kt, p_t, qt)], oldest first
            pending_norm = None

            def flush_pv_group():
                group = pending_pv.pop(0)
                for b, h, pkt, p_ap, pqt in group:
                    o = pkt * HVW + h * (DH + 1)
                    nc.tensor.matmul(
                        ctx_map[pqt][b][:, h, :],
                        hv_sb[b][:, o : o + 128],
                        p_ap,
                        start=(pkt == 0),
                        stop=(pkt == KN - 1),
                    )

            out_parts = []  # deferred output-projection closures

            def emit_out_part(b, pqt, qs, tail=False):
                ctxn = ctxn_sb[(b, pqt)]
                ob = outp.tile([128, D], F16, tag="outb", name=f"ob{b}_{qs}_{pqt}")
                op_ps = psp.tile(
                    [128, 2, QTS], F32, tag="sc", name=f"op{b}_{qs}_{pqt}"
                )
                for oh in range(2):
                    nc.tensor.matmul(
                        op_ps[:, oh, :],
                        ctxn[:, qs * 128 : (qs + 1) * 128],
                        wo_t[:, oh * QTS : (oh + 1) * QTS],
                        start=True,
                        stop=True,
                    )
                if tail and qs % 2 == 1:
                    # ACT is idle at the tail; split the PSUM-drain load and
                    # use both DMA queues (no exp stream left to block)
                    nc.scalar.copy(ob[:], op_ps[:])
                    nc.scalar.dma_start(
                        out[b, pqt * QTS + qs * 128 : pqt * QTS + qs * 128 + 128, :],
                        ob[:],
                    )
                else:
                    nc.vector.tensor_copy(ob[:], op_ps[:])
                    nc.sync.dma_start(
                        out[b, pqt * QTS + qs * 128 : pqt * QTS + qs * 128 + 128, :],
                        ob[:],
                    )

            def emit_qk(qt_, kt_):
                # QK: (h0, h1) into a 2-bank PSUM tile (h0 rows 0-63 /
                # h1 rows 64-127 also land on different PE row-tiles)
                sc_ = {}
                for b in range(B):
                    sc_[b] = psp.tile(
                        [128, HPC, QTS],
                        F32,
                        tag="sc",
                        name=f"sc{b}_{kt_}_{qt_}",
                    )
                    for h in range(HPC):
                        nc.tensor.matmul(
                            sc_[b][:, h, :],
                            hk_sb[b][
                                h * DH : (h + 1) * DH, kt_ * KP : (kt_ + 1) * KP
                            ],
                            hq_sb[b][
                                h * DH : (h + 1) * DH,
                                qt_ * QTS : (qt_ + 1) * QTS,
                            ],
                            start=True,
                            stop=True,
                        )
                return sc_

            # QK emission runs one k-tile AHEAD of exp/PV emission, so each
            # next tile's QK sits in the PE queue BEFORE the current tile's
            # PV flush and output-projection matmuls: the exp stream never
            # waits on tail-of-tile PE work.
            tl = [(qt, kt) for qt in range(QN) for kt in range(KN)]
            eb_cur = None
            sc_cur = None
            for idx, (qt, kt) in enumerate(tl):
                kg, ki = kt // KTG, kt % KTG
                just_norm = False
                if kt == 0:
                    ctx_map[qt] = {
                        bb: psp.tile(
                            [128, HPC, QTS],
                            F32,
                            tag="ctx",
                            name=f"ctx_ps{bb}_{qt}",
                        )
                        for bb in range(B)
                    }
                if ki == 0:
                    if qt == 0 and kg == 0:
                        eb_cur = pre_eb
                    else:
                        eb_cur = ebp.tile(
                            [128, KTG, B, HPC, QTS], F16, tag="eb", name=f"eb_t{qt}_{kg}"
                        )
                        nc.gpsimd.dma_start(eb_cur[:], eb[qt, kg])
                if idx == 0:
                    sc_cur = emit_qk(qt, kt)
                sc_nxt = emit_qk(*tl[idx + 1]) if idx + 1 < len(tl) else None
                # both batches' probs in one tile: 2 exps, ONE wide
                # fp16 multiply (DVE 2x mode over 2048 columns)
                p1_t = p1p.tile(
                    [128, B * HPC, QTS], F16, tag="p1", name=f"p1_{kt}_{qt}"
                )
                for b in range(B):
                    nc.scalar.activation(
                        p1_t[:, b * HPC : (b + 1) * HPC, :],
                        sc_cur[b][:],
                        mybir.ActivationFunctionType.Exp,
                    )
                p2 = ptp.tile(
                    [128, B * HPC, QTS],
                    F16,
                    tag="pt2",
                    bufs=7,
                    name=f"p2_{kt}_{qt}",
                )
                nc.vector.tensor_tensor(
                    p2[:],
                    p1_t[:],
                    eb_cur[:, ki, :, :, :],
                    mybir.AluOpType.mult,
                )
                new_group = []
                for b in range(B):
                    for h in range(HPC):
                        new_group.append((b, h, kt, p2[:, b * HPC + h, :], qt))
                pending_pv.append(new_group)
                sc_cur = sc_nxt
                if kt == PV_LAG and pending_norm is not None:
                    # all of qt-1's PVs have flushed; normalize it (frees
                    # its ctx banks) and queue its output projections
                    pqt = pending_norm[0]
                    emit_normalize(*pending_norm)
                    pending_norm = None
                    for bb in range(B):
                        for qs in range(QN):
                            out_parts.append((bb, pqt, qs))
                    just_norm = True
                if just_norm:
                    pass  # skip one flush beat: give the normalize
                    # chain time before the first new-ctx PV
                else:
                    while len(pending_pv) > PV_LAG:
                        flush_pv_group()
                if (
                    out_parts
                    and kt >= PV_LAG + 1
                    and (kt % 2 == 0 or len(out_parts) > 6)
                ):
                    emit_out_part(*out_parts.pop(0))
                if kt == KN - 1:
                    pending_norm = (qt, ctx_map[qt])
            while pending_pv:
                flush_pv_group()
            pqt = pending_norm[0]
            emit_normalize(*pending_norm)
            for bb in range(B):
                for qs in range(QN):
                    out_parts.append((bb, pqt, qs))
            while out_parts:
                emit_out_part(*out_parts.pop(0), tail=True)

    nc.compile()
    _CACHE["nc"] = nc
    return nc


def _prep_core(core, position_bias, Wq, Wk, Wv, Wo, shared):
    """Per-core input map. `shared` holds core-independent packed arrays."""
    h0 = core * HPC
    rows = slice(h0 * DH, (h0 + HPC) * DH)

    def packw(w, scale=1.0):
        return np.ascontiguousarray(
            (w[rows].T * scale).reshape(DC, 128, 128).transpose(1, 0, 2)
        ).astype(np.float16)

    # E = exp(pb) * keep, per (b, h) -> [qt, kg, kp, ktg, b, h, qf]
    ecomb = shared["epb"][h0 : h0 + HPC][None] * shared["keep"][:, None]  # [B,HPC,q,k]
    ebp = np.ascontiguousarray(
        ecomb.reshape(B, HPC, QN, QTS, KGN, KTG, 128).transpose(2, 4, 6, 5, 0, 1, 3)
    ).astype(np.float16)
    return {
        "qT": shared["qT"],
        "kvT": shared["kvT"],
        "identr": shared["identr"],
        "indh": shared["indh"],
        "wq": packw(Wq, 1.0 / np.sqrt(DH)),
        "wk": packw(Wk),
        "wv": packw(Wv),
        "wo": np.ascontiguousarray(Wo[:, rows].T).astype(np.float16),
        "eb": ebp,
    }


def _prep_shared(query, key_value, mask, position_bias):
    qTp = np.ascontiguousarray(
        query.reshape(B, L, DC, 128).transpose(0, 2, 3, 1)
    ).astype(np.float16)
    kvTp = np.ascontiguousarray(
        key_value.reshape(B, L, DC, 128).transpose(0, 2, 3, 1)
    ).astype(np.float16)
    epb = np.exp(position_bias, dtype=np.float32)  # [H, q, k]
    keep = np.asarray(mask, dtype=np.float32)  # [B, q, k] 1=keep
    indh = np.concatenate(
        [
            np.where(np.arange(128) < 64, 1.0, 0.0),
            np.where(np.arange(128) >= 64, 1.0, 0.0),
        ]
    ).astype(np.float16)[None, :]
    return {
        "qT": qTp,
        "kvT": kvTp,
        "epb": epb,
        "keep": keep,
        "identr": np.eye(128, dtype=np.float32),
        "indh": np.ascontiguousarray(indh),
    }


def kernel(query, key_value, mask, position_bias, Wq, Wk, Wv, Wo, _trace=False):
    query = np.asarray(query, dtype=np.float32)
    key_value = np.asarray(key_value, dtype=np.float32)
    mask = np.asarray(mask)
    position_bias = np.asarray(position_bias, dtype=np.float32)
    Wq = np.asarray(Wq, dtype=np.float32)
    Wk = np.asarray(Wk, dtype=np.float32)
    Wv = np.asarray(Wv, dtype=np.float32)
    Wo = np.asarray(Wo, dtype=np.float32)

    nc = _build()
    shared = _prep_shared(query, key_value, mask, position_bias)
    in_maps = [
        _prep_core(c, position_bias, Wq, Wk, Wv, Wo, shared) for c in range(N_CORES)
    ]
    res = run_bass_kernel_spmd(nc, in_maps, list(range(N_CORES)), trace=_trace)
    _CACHE["last_result"] = res
    acc = res.results[0]["out"].astype(np.float64)
    for c in range(1, N_CORES):
        acc += res.results[c]["out"]
    return acc.astype(np.float32)


# revision 39
# speedup vs baseline: 1.0946x; 1.0165x over previous
"""CPM3 attention kernel for 8 trn2 NeuronCores.

Sharding: tensor-parallel over heads (2 heads/core x both batches).
Device computes per-core partial outputs (Wo row-sharded); host sums.

Data layout tricks:
- host pre-transposes q/kv so the device never transposes big tensors;
  scores are computed transposed [k, q] so the softmax needs no
  partition-dim reductions (a ones-column in V yields the denominators).
- fp16 operands for all matmuls; PSUM accumulation stays fp32.
- softmax bias/mask enter MULTIPLICATIVELY: host precomputes
  E = exp(position_bias) * keep_mask, so p = exp(qk) * E.
- QK scores for (h0, h1) land in one 2-bank PSUM tile, so a single
  ACT instruction exps 1024 columns (amortizes ACT instruction
  overhead); PV runs 2 k-tiles behind QK (software pipeline).
- epilogue split: only the softmax normalization happens at q-tile
  boundaries; all output projections run as one dense tail block so
  they never stall the main-loop pipeline.
- PSUM budget: tag "sc" = 2 tiles x 2 banks, tag "ctx" = 2 tiles x 2
  banks; prologue/epilogue tiles reuse the same rings.
"""

import sys

sys.path.insert(0, "/opt/trn_rl_repo")

import numpy as np
import ml_dtypes

import concourse.bass as bass
import concourse.bacc as bacc
import concourse.tile as tile
import concourse.mybir as mybir
from concourse.bass_utils import run_bass_kernel_spmd

B, L, D, H, DH = 2, 2048, 1024, 16, 64
N_CORES = 8
HPC = H // N_CORES  # heads per core = 2
QTS = 512  # q tile size
QN = L // QTS  # 4
KP = 128  # k partition tile
KN = L // KP  # 16
KTG = 4  # k tiles per DMA group
KGN = KN // KTG  # 4
DC = D // 128  # 8 contraction chunks
HVW = 2 * (DH + 1)  # 130: hv_aug columns per k-tile (2 heads x (64+ones))
PV_LAG = 4  # k-tiles of software-pipeline distance for PV

F32 = mybir.dt.float32
F32R = mybir.dt.float32r
F16 = mybir.dt.float16

_CACHE: dict = {}


def _build():
    if "nc" in _CACHE:
        return _CACHE["nc"]
    nc = bacc.Bacc("TRN2", target_bir_lowering=False, debug=False, num_devices=N_CORES)

    qT = nc.dram_tensor("qT", [B, DC, 128, L], F16, kind="ExternalInput").ap()
    kvT = nc.dram_tensor("kvT", [B, DC, 128, L], F16, kind="ExternalInput").ap()
    wq = nc.dram_tensor("wq", [128, DC, 128], F16, kind="ExternalInput").ap()
    wk = nc.dram_tensor("wk", [128, DC, 128], F16, kind="ExternalInput").ap()
    wv = nc.dram_tensor("wv", [128, DC, 128], F16, kind="ExternalInput").ap()
    wo = nc.dram_tensor("wo", [128, D], F16, kind="ExternalInput").ap()
    eb = nc.dram_tensor(
        "eb", [QN, KGN, 128, KTG, B, HPC, QTS], F16, kind="ExternalInput"
    ).ap()
    identr = nc.dram_tensor("identr", [128, 128], F32R, kind="ExternalInput").ap()
    indh = nc.dram_tensor("indh", [1, 256], F16, kind="ExternalInput").ap()
    out = nc.dram_tensor("out", [B, L, D], F16, kind="ExternalOutput").ap()

    with tile.TileContext(nc) as tc:
        with (
            tc.tile_pool(name="const", bufs=1) as constp,
            tc.tile_pool(name="hq", bufs=2) as hqp,
            tc.tile_pool(name="hk", bufs=2) as hkp,
            tc.tile_pool(name="hv", bufs=2) as hvp,
            tc.tile_pool(name="stage", bufs=3) as stagep,
            tc.tile_pool(name="ebp", bufs=2) as ebp,
            tc.tile_pool(name="p1", bufs=6) as p1p,
            tc.tile_pool(name="pt", bufs=14) as ptp,
            tc.tile_pool(name="ctxn", bufs=2 * QN) as ctxnp,
            tc.tile_pool(name="rc", bufs=4) as rcp,
            tc.tile_pool(name="outb", bufs=4) as outp,
            tc.tile_pool(name="psum", bufs=2, space=bass.MemorySpace.PSUM) as psp,
        ):
            # ---- constants (wq/wk first: the prologue blocks on them) ----
            wq_t = constp.tile([128, DC, 128], F16, tag="wq")
            nc.sync.dma_start(wq_t[:], wq[:])
            wk_t = constp.tile([128, DC, 128], F16, tag="wk")
            nc.scalar.dma_start(wk_t[:], wk[:])
            wv_t = constp.tile([128, DC, 128], F16, tag="wv")
            nc.scalar.dma_start(wv_t[:], wv[:])
            identr_t = constp.tile([128, 128], F32R, tag="identr")
            nc.sync.dma_start(identr_t[:], identr[:])
            indh_t = constp.tile([1, 256], F16, tag="indh")
            nc.sync.dma_start(indh_t[:], indh[:])
            wo_t = constp.tile([128, D], F16, tag="wo")
            nc.sync.dma_start(wo_t[:], wo[:])

            # ---- prologue: projections ----
            # q chunks on the sync queue, kv chunks on the scalar queue (more
            # SDMA parallelism); each [128, L] chunk is loaded as two halves
            # so matmuls start after the first half arrives.
            hq_sb, hk_sb, hv_sb, hvT = {}, {}, {}, {}
            hq_ps, hk_ps, hv_ps = {}, {}, {}

            def half_dma(engines, dst, src):
                engines[0].dma_start(dst[:, 0 : L // 2], src[:, 0 : L // 2])
                engines[1].dma_start(dst[:, L // 2 : L], src[:, L // 2 : L])

            for b in range(B):
                # -- projection matmuls for batch b --
                hq_ps[b] = [
                    psp.tile([128, 2, QTS], F32, tag="sc", name=f"hq_ps{b}_{i}")
                    for i in range(2)
                ]
                for dc in range(DC):
                    qc = stagep.tile([128, L], F16, tag="stage", name=f"qc{b}_{dc}")
                    half_dma((nc.sync, nc.sync), qc, qT[b, dc])
                    for qt in range(QN):
                        nc.tensor.matmul(
                            hq_ps[b][qt // 2][:, qt % 2, :],
                            wq_t[:, dc, :],
                            qc[:, qt * QTS : (qt + 1) * QTS],
                            start=(dc == 0),
                            stop=(dc == DC - 1),
                        )
                hq_sb[b] = hqp.tile([128, L], F16, tag="hq", name=f"hq_sb{b}")
                for qt in range(QN):
                    nc.scalar.copy(
                        hq_sb[b][:, qt * QTS : (qt + 1) * QTS],
                        hq_ps[b][qt // 2][:, qt % 2, :],
                    )

                hk_ps[b] = [
                    psp.tile([128, 2, QTS], F32, tag="sc", name=f"hk_ps{b}_{i}")
                    for i in range(2)
                ]
                hv_ps[b] = [
                    psp.tile([128, 2, QTS], F32, tag="ctx", name=f"hv_ps{b}_{i}")
                    for i in range(2)
                ]
                for dc in range(DC):
                    kc = stagep.tile([128, L], F16, tag="stage", name=f"kc{b}_{dc}")
                    half_dma((nc.scalar, nc.gpsimd), kc, kvT[b, dc])
                    for qt in range(QN):
                        nc.tensor.matmul(
                            hk_ps[b][qt // 2][:, qt % 2, :],
                            wk_t[:, dc, :],
                            kc[:, qt * QTS : (qt + 1) * QTS],
                            start=(dc == 0),
                            stop=(dc == DC - 1),
                        )
                        nc.tensor.matmul(
                            hv_ps[b][qt // 2][:, qt % 2, :],
                            wv_t[:, dc, :],
                            kc[:, qt * QTS : (qt + 1) * QTS],
                            start=(dc == 0),
                            stop=(dc == DC - 1),
                        )
                hk_sb[b] = hkp.tile([128, L], F16, tag="hk", name=f"hk_sb{b}")
                hvT[b] = stagep.tile([128, L], F32R, tag="stage", name=f"hvT{b}")
                for qt in range(QN):
                    nc.scalar.copy(
                        hk_sb[b][:, qt * QTS : (qt + 1) * QTS],
                        hk_ps[b][qt // 2][:, qt % 2, :],
                    )
                    nc.vector.tensor_copy(
                        hvT[b][:, qt * QTS : (qt + 1) * QTS],
                        hv_ps[b][qt // 2][:, qt % 2, :],
                    )

            # E prefetch: issued only now so the projection DMAs get the
            # full HBM bandwidth first
            pre_eb = ebp.tile([128, KTG, B, HPC, QTS], F16, tag="eb", name="pre_eb")
            nc.gpsimd.dma_start(pre_eb[:], eb[0, 0])

            # -- hv_aug: transpose hvT per k-tile; ones cols prefilled --
            for b in range(B):
                hv_sb[b] = hvp.tile(
                    [128, KN * HVW + 64], F16, tag="hv", name=f"hv_sb{b}"
                )
                nc.gpsimd.memset(hv_sb[b][:].bitcast(mybir.dt.uint16), 0x3C00)
            for b in range(B):
                for kt in range(KN):
                    tp = psp.tile([128, 128], F32R, tag="sc")
                    nc.tensor.transpose(
                        tp[:], hvT[b][:, kt * KP : (kt + 1) * KP], identr_t[:]
                    )
                    o = kt * HVW
                    nc.vector.tensor_copy(hv_sb[b][:, o : o + DH], tp[:, 0:DH])
                    nc.vector.tensor_copy(
                        hv_sb[b][:, o + DH + 1 : o + 2 * DH + 1], tp[:, DH:128]
                    )

            # ---- normalize: softmax denominators -> ctxn (fp16) ----
            ctxn_sb = {}

            def emit_normalize(qt, ctx_ps):
                for b in range(B):
                    ctxn = ctxnp.tile(
                        [128, QTS], F16, tag="ctxn", name=f"ctxn{b}_{qt}"
                    )
                    ctxn_sb[(b, qt)] = ctxn
                    bcw = psp.tile([128, 2, QTS], F32, tag="sc", name=f"bcw{b}_{qt}")
                    bc = bcw[:, 0, :]
                    for h in range(HPC):
                        dsb = rcp.tile(
                            [1, QTS], F32, tag="dsb", name=f"dsb{b}_{h}_{qt}"
                        )
                        nc.vector.tensor_copy(dsb[:], ctx_ps[b][DH : DH + 1, h, :])
                        rcf = rcp.tile(
                            [1, QTS], F32, tag="rcf", name=f"rcf{b}_{h}_{qt}"
                        )
                        nc.vector.reciprocal_approx_fast(rcf[:], dsb[:])
                        rcr = rcp.tile(
                            [1, QTS], F16, tag="rcr", name=f"rcr{b}_{h}_{qt}"
                        )
                        nc.vector.tensor_copy(rcr[:], rcf[:])
                        nc.tensor.matmul(
                            bc,
                            indh_t[:, h * 128 : (h + 1) * 128],
                            rcr[:],
                            start=(h == 0),
                            stop=(h == HPC - 1),
                        )
                    bc_sb = rcp.tile([128, QTS], F32, tag="bcsb", name=f"bc_sb{b}_{qt}")
                    nc.vector.tensor_copy(bc_sb[:], bc)
                    for h in range(HPC):
                        nc.vector.tensor_tensor(
                            ctxn[h * DH : (h + 1) * DH, :],
                            ctx_ps[b][0:DH, h, :],
                            bc_sb[h * DH : (h + 1) * DH, :],
                            mybir.AluOpType.mult,
                        )

            # ---- main loop ----
            ctx_map = {}  # qt -> {b: [128, HPC, QTS] psum ap}
            pending_pv = []  # groups of [(b, h, kt, p_t, qt)], oldest first
            pending_norm = None

            def flush_pv_group():
                group = pending_pv.pop(0)
                for b, h, pkt, p_ap, pqt in group:
                    o = pkt * HVW + h * (DH + 1)
                    nc.tensor.matmul(
                        ctx_map[pqt][b][:, h, :],
                        hv_sb[b][:, o : o + 128],
                        p_ap,
                        start=(pkt == 0),
                        stop=(pkt == KN - 1),
                    )

            out_parts = []  # deferred output-projection closures

            def emit_out_part(b, pqt, qs, tail=False):
                ctxn = ctxn_sb[(b, pqt)]
                ob = outp.tile([128, D], F16, tag="outb", name=f"ob{b}_{qs}_{pqt}")
                op_ps = psp.tile(
                    [128, 2, QTS], F32, tag="sc", name=f"op{b}_{qs}_{pqt}"
                )
                for oh in range(2):
                    nc.tensor.matmul(
                        op_ps[:, oh, :],
                        ctxn[:, qs * 128 : (qs + 1) * 128],
                        wo_t[:, oh * QTS : (oh + 1) * QTS],
                        start=True,
                        stop=True,
                    )
                if tail and qs % 2 == 1:
                    # ACT is idle at the tail; split the PSUM-drain load and
                    # use both DMA queues (no exp stream left to block)
                    nc.scalar.copy(ob[:], op_ps[:])
                    nc.scalar.dma_start(
                        out[b, pqt * QTS + qs * 128 : pqt * QTS + qs * 128 + 128, :],
                        ob[:],
                    )
                else:
                    nc.vector.tensor_copy(ob[:], op_ps[:])
                    nc.sync.dma_start(
                        out[b, pqt * QTS + qs * 128 : pqt * QTS + qs * 128 + 128, :],
                        ob[:],
                    )

            def emit_qk(qt_, kt_):
                # QK: (h0, h1) into a 2-bank PSUM tile (h0 rows 0-63 /
                # h1 rows 64-127 also land on different PE row-tiles)
                sc_ = {}
                for b in range(B):
                    sc_[b] = psp.tile(
                        [128, HPC, QTS],
                        F32,
                        tag="sc",
                        name=f"sc{b}_{kt_}_{qt_}",
                    )
                    for h in range(HPC):
                        nc.tensor.matmul(
                            sc_[b][:, h, :],
                            hk_sb[b][
                                h * DH : (h + 1) * DH, kt_ * KP : (kt_ + 1) * KP
                            ],
                            hq_sb[b][
                                h * DH : (h + 1) * DH,
                                qt_ * QTS : (qt_ + 1) * QTS,
                            ],
                            start=True,
                            stop=True,
                        )
                return sc_

            # QK emission runs one k-tile AHEAD of exp/PV emission, so each
            # next tile's QK sits in the PE queue BEFORE the current tile's
            # PV flush and output-projection matmuls: the exp stream never
            # waits on tail-of-tile PE work.
            tl = [(qt, kt) for qt in range(QN) for kt in range(KN)]
            eb_cur = None
            sc_cur = None
            for idx, (qt, kt) in enumerate(tl):
                kg, ki = kt // KTG, kt % KTG
                just_norm = False
                if kt == 0:
                    ctx_map[qt] = {
                        bb: psp.tile(
                            [128, HPC, QTS],
                            F32,
                            tag="ctx",
                            name=f"ctx_ps{bb}_{qt}",
                        )
                        for bb in range(B)
                    }
                if ki == 0:
                    if qt == 0 and kg == 0:
                        eb_cur = pre_eb
                    else:
                        eb_cur = ebp.tile(
                            [128, KTG, B, HPC, QTS], F16, tag="eb", name=f"eb_t{qt}_{kg}"
                        )
                        nc.gpsimd.dma_start(eb_cur[:], eb[qt, kg])
                if idx == 0:
                    sc_cur = emit_qk(qt, kt)
                sc_nxt = emit_qk(*tl[idx + 1]) if idx + 1 < len(tl) else None
                if kt == PV_LAG and pending_norm is not None:
                    # all of qt-1's PVs have flushed; normalize it now so the
                    # DVE recip chain (which gates the bcw sc-ring slot that
                    # a QK two tiles later reuses) starts as early as possible
                    pqt = pending_norm[0]
                    emit_normalize(*pending_norm)
                    pending_norm = None
                    for bb in range(B):
                        for qs in range(QN):
                            out_parts.append((bb, pqt, qs))
                    just_norm = True
                if (
                    out_parts
                    and kt >= PV_LAG + 1
                    and (kt % 2 == 0 or len(out_parts) > 6)
                ):
                    # part's PSUM drain leads the DVE queue so its borrowed
                    # sc-ring slot frees before the QK two tiles later
                    emit_out_part(*out_parts.pop(0))
                # both batches' probs in one tile: 2 exps, ONE wide
                # fp16 multiply (DVE 2x mode over 2048 columns)
                p1_t = p1p.tile(
                    [128, B * HPC, QTS], F16, tag="p1", name=f"p1_{kt}_{qt}"
                )
                for b in range(B):
                    nc.scalar.activation(
                        p1_t[:, b * HPC : (b + 1) * HPC, :],
                        sc_cur[b][:],
                        mybir.ActivationFunctionType.Exp,
                    )
                p2 = ptp.tile(
                    [128, B * HPC, QTS],
                    F16,
                    tag="pt2",
                    bufs=7,
                    name=f"p2_{kt}_{qt}",
                )
                nc.vector.tensor_tensor(
                    p2[:],
                    p1_t[:],
                    eb_cur[:, ki, :, :, :],
                    mybir.AluOpType.mult,
                )
                new_group = []
                for b in range(B):
                    for h in range(HPC):
                        new_group.append((b, h, kt, p2[:, b * HPC + h, :], qt))
                pending_pv.append(new_group)
                sc_cur = sc_nxt
                if just_norm:
                    pass  # skip one flush beat: give the normalize
                    # chain time before the first new-ctx PV
                else:
                    while len(pending_pv) > PV_LAG:
                        flush_pv_group()
                if kt == KN - 1:
                    pending_norm = (qt, ctx_map[qt])
            while pending_pv:
                flush_pv_group()
            pqt = pending_norm[0]
            emit_normalize(*pending_norm)
            for bb in range(B):
                for qs in range(QN):
                    out_parts.append((bb, pqt, qs))
            while out_parts:
                emit_out_part(*out_parts.pop(0), tail=True)

    nc.compile()
    _CACHE["nc"] = nc
    return nc


def _prep_core(core, position_bias, Wq, Wk, Wv, Wo, shared):
    """Per-core input map. `shared` holds core-independent packed arrays."""
    h0 = core * HPC
    rows = slice(h0 * DH, (h0 + HPC) * DH)

    def packw(w, scale=1.0):
        return np.ascontiguousarray(
            (w[rows].T * scale).reshape(DC, 128, 128).transpose(1, 0, 2)
        ).astype(np.float16)

    # E = exp(pb) * keep, per (b, h) -> [qt, kg, kp, ktg, b, h, qf]
    ecomb = shared["epb"][h0 : h0 + HPC][None] * shared["keep"][:, None]  # [B,HPC,q,k]
    ebp = np.ascontiguousarray(
        ecomb.reshape(B, HPC, QN, QTS, KGN, KTG, 128).transpose(2, 4, 6, 5, 0, 1, 3)
    ).astype(np.float16)
    return {
        "qT": shared["qT"],
        "kvT": shared["kvT"],
        "identr": shared["identr"],
        "indh": shared["indh"],
        "wq": packw(Wq, 1.0 / np.sqrt(DH)),
        "wk": packw(Wk),
        "wv": packw(Wv),
        "wo": np.ascontiguousarray(Wo[:, rows].T).astype(np.float16),
        "eb": ebp,
    }


def _prep_shared(query, key_value, mask, position_bias):
    qTp = np.ascontiguousarray(
        query.reshape(B, L, DC, 128).transpose(0, 2, 3, 1)
    ).astype(np.float16)
    kvTp = np.ascontiguousarray(
        key_value.reshape(B, L, DC, 128).transpose(0, 2, 3, 1)
    ).astype(np.float16)
    epb = np.exp(position_bias, dtype=np.float32)  # [H, q, k]
    keep = np.asarray(mask, dtype=np.float32)  # [B, q, k] 1=keep
    indh = np.concatenate(
        [
            np.where(np.arange(128) < 64, 1.0, 0.0),
            np.where(np.arange(128) >= 64, 1.0, 0.0),
        ]
    ).astype(np.float16)[None, :]
    return {
        "qT": qTp,
        "kvT": kvTp,
        "epb": epb,
        "keep": keep,
        "identr": np.eye(128, dtype=np.float32),
        "indh": np.ascontiguousarray(indh),
    }


def kernel(query, key_value, mask, position_bias, Wq, Wk, Wv, Wo, _trace=False):
    query = np.asarray(query, dtype=np.float32)
    key_value = np.asarray(key_value, dtype=np.float32)
    mask = np.asarray(mask)
    position_bias = np.asarray(position_bias, dtype=np.float32)
    Wq = np.asarray(Wq, dtype=np.float32)
    Wk = np.asarray(Wk, dtype=np.float32)
    Wv = np.asarray(Wv, dtype=np.float32)
    Wo = np.asarray(Wo, dtype=np.float32)

    nc = _build()
    shared = _prep_shared(query, key_value, mask, position_bias)
    in_maps = [
        _prep_core(c, position_bias, Wq, Wk, Wv, Wo, shared) for c in range(N_CORES)
    ]
    res = run_bass_kernel_spmd(nc, in_maps, list(range(N_CORES)), trace=_trace)
    _CACHE["last_result"] = res
    acc = res.results[0]["out"].astype(np.float64)
    for c in range(1, N_CORES):
        acc += res.results[c]["out"]
    return acc.astype(np.float32)
